# revision 19
# baseline (speedup 1.0000x reference)
"""Trainium2 Bass kernel for nn_DAE_44779329028610 (embedding autoencoder).

  y = sigmoid(sigmoid(x @ w + b) @ w.T)
  x [4096, 81616] f32, w [81616, 32] f32, b [32] f32 -> y [4096, 81616] f32

Strategy: data-parallel shard of the batch dim across 8 NeuronCores
(512 rows/core); w replicated. The workload is HBM-bound, so the kernel
moves all bulk tensors in bf16 (rel-err budget is 2e-2; bf16 keeps it
~1e-3): the host pre-converts x to bf16 and pre-packs two device-friendly
w layouts (encoder layout [128, 638, 32] = w rows scattered mod 128, and
the decoder's transposed layout [128, 20480] = w.T split into 4
partition-group quarters), and y is produced in bf16 and widened on the
host. This roughly halves HBM traffic vs f32 (167+167 MB -> 84+84 MB per
core) and removes all on-device w transposition.

Per core the kernel is batch-tile pipelined: for each of the 4 tiles of
128 batch rows it encodes (stream x [128, S]-chunks; PE-transpose each
128x128 block into PSUM; evict to bf16 SBUF on DVE/Pool; accumulate
hT[32, 128] over all 638 vocab chunks with the w-chunk [128v, 32] as
stationary) and then decodes (hT sigmoid+bias on ACT, replicated to the
4 PE row groups; K=32 matmuls against the resident wT quarters at
tile_position=(32g, 0); ACT applies sigmoid PSUM->bf16 SBUF; y leaves in
[128, 4096] DMAs). Decode of tile t overlaps encode of tile t+1, so
x-read and y-write DMA streams stay concurrently busy and the ACT
sigmoid work hides under them. x reads issue on the SP DMA queue and
y writes on the ACT queue to avoid head-of-line blocking between the
two streams.
"""

import sys

if "/opt/trn_rl_repo" not in sys.path:
    sys.path.insert(0, "/opt/trn_rl_repo")

from contextlib import ExitStack

import ml_dtypes
import numpy as np

from concourse import bacc, masks, mybir, tile
from concourse.bass_utils import run_bass_kernel_spmd

# The neuronx_cc hook recompiles the NEFF from scratch in every process
# (~minutes of walrus for this kernel). Cache the compiled NEFF on disk,
# keyed by the BIR hash, so repeat runs are instant.
import hashlib
import os
import shutil

import concourse.bass2jax as _bass2jax

_NEFF_CACHE_DIR = "/tmp/bass_neff_cache"
_orig_compile_bir_kernel = _bass2jax.compile_bir_kernel


def _cached_compile_bir_kernel(bir_json, tmpdir, neff_name="file.neff"):
    os.makedirs(_NEFF_CACHE_DIR, exist_ok=True)
    key = hashlib.sha256(bir_json).hexdigest()[:32]
    cpath = os.path.join(_NEFF_CACHE_DIR, f"{key}.neff")
    out = os.path.join(tmpdir, neff_name)
    if os.path.exists(cpath):
        shutil.copyfile(cpath, out)
        return out
    out = _orig_compile_bir_kernel(bir_json, tmpdir, neff_name)
    try:
        shutil.copyfile(out, cpath)
    except OSError:
        pass
    return out


_bass2jax.compile_bir_kernel = _cached_compile_bir_kernel

F32 = mybir.dt.float32
BF16 = mybir.dt.bfloat16
BF16_NP = ml_dtypes.bfloat16
SIG = mybir.ActivationFunctionType.Sigmoid

B_FULL = 4096
V = 81616
D = 32
N_CORES = 8
B_CORE = B_FULL // N_CORES
NCH = -(-V // 128)  # 638 vocab chunks of 128
VPAD = NCH * 128  # 81664
QB = 20480  # wT quarter width (vocab cols per 32-partition group)


def build_dae(
    B_core=B_CORE,
    V=V,
    S=8192,
    YS=4096,
    x_bufs=3,
    y_bufs=4,
    y_on_sp=False,
    y_defer=0,
    pace=1.0,
    repeat=1,
):
    """Build + compile the per-core Bass program. S = x stream chunk,
    YS = y store chunk (one DMA each)."""
    assert B_core % 128 == 0
    nbt = B_core // 128
    assert S % 512 == 0 and YS % 1024 == 0

    nc = bacc.Bacc("TRN2", target_bir_lowering=False, debug=False)

    x_d = nc.dram_tensor("x", [B_core, V], BF16, kind="ExternalInput")
    wenc_d = nc.dram_tensor("wenc", [128, NCH * D], BF16, kind="ExternalInput")
    wt_d = nc.dram_tensor("wt", [128, QB], BF16, kind="ExternalInput")
    b_d = nc.dram_tensor("b", [D], F32, kind="ExternalInput")
    y_d = nc.dram_tensor("y", [B_core, V], BF16, kind="ExternalOutput")

    with tile.TileContext(nc) as tc, ExitStack() as ctx:
        const_pool = ctx.enter_context(tc.tile_pool(name="const", bufs=1))
        ident = const_pool.tile([128, 128], BF16)
        masks.make_identity(nc, ident[:])
        b_sb = const_pool.tile([D, 1], F32)
        nc.sync.dma_start(b_sb[:, 0:1], b_d[:].unsqueeze(1))
        # persistent weights: encoder layout + transposed decoder layout
        # (loaded in pieces interleaved with the first x stream -- see _body)
        w_sb = const_pool.tile([128, NCH, D], BF16)
        wt_sb = const_pool.tile([128, QB], BF16)

        def _body():
            y_eng = nc.sync if y_on_sp else nc.scalar
            with ExitStack() as es:
                xpool = es.enter_context(tc.tile_pool(name="x", bufs=x_bufs))
                xtps = es.enter_context(tc.tile_pool(name="xtps", bufs=3, space="PSUM"))
                xtsb = es.enter_context(tc.tile_pool(name="xtsb", bufs=4))
                htps = es.enter_context(tc.tile_pool(name="htps", bufs=1, space="PSUM"))
                htsb = es.enter_context(tc.tile_pool(name="htsb", bufs=2))
                yps = es.enter_context(tc.tile_pool(name="yps", bufs=2, space="PSUM"))
                ysb = es.enter_context(tc.tile_pool(name="ysb", bufs=y_bufs))

                def _emit_accums(pending, ht_ps):
                    xt_sb, subs = pending
                    for ci, j, vlen in subs:
                        nc.tensor.matmul(
                            ht_ps[:, :],
                            w_sb[0:vlen, ci, :],
                            xt_sb[0:vlen, j * 128 : (j + 1) * 128],
                            start=(ci == 0),
                            stop=(ci == NCH - 1),
                        )

                def _decode_units(t, ht_sb):
                    """Decode tile t as a list of closures, each: 2 matmuls
                    into a fresh y_ps + 1 ACT sigmoid (+ y DMA on the last
                    piece of a y_sb). Interleaved into the NEXT tile's encode
                    emission so the in-order PE queue never couples encode
                    progress to the ACT-paced PSUM drain."""
                    r0 = t * 128
                    units = []
                    dma_after = {}  # unit index -> y DMA closure (deferred)
                    for g in range(4):
                        q0 = g * QB
                        qvalid = min(QB, V - q0)
                        for yb0 in range(0, qvalid, YS):
                            wlen = min(YS, qvalid - yb0)
                            ns0 = -(-wlen // 1024) * 1024
                            cell = {}
                            for s0 in range(0, ns0, 1024):
                                def u(g=g, yb0=yb0, s0=s0, cell=cell, ht_sb=ht_sb):
                                    if s0 == 0:
                                        cell["y_sb"] = ysb.tile(
                                            [128, YS], BF16, name="y_sb"
                                        )
                                    y_sb = cell["y_sb"]
                                    y_ps = yps.tile([128, 1024], F32)
                                    for m0 in range(0, 1024, 512):
                                        nc.tensor.matmul(
                                            y_ps[:, m0 : m0 + 512],
                                            ht_sb[32 * g : 32 * g + D, :],
                                            wt_sb[
                                                32 * g : 32 * g + D,
                                                yb0 + s0 + m0 : yb0 + s0 + m0 + 512,
                                            ],
                                            tile_position=(32 * g, 0),
                                        )
                                    nc.scalar.activation(
                                        y_sb[:, s0 : s0 + 1024], y_ps[:, :], SIG
                                    )
                                units.append(u)
                            # y write issued a few units after its last
                            # sigmoid so its semaphore wait is already
                            # satisfied when the sequencer reaches it
                            def ydma(q0=q0, yb0=yb0, wlen=wlen, cell=cell, r0=r0):
                                y_eng.dma_start(
                                    y_d[r0 : r0 + 128, q0 + yb0 : q0 + yb0 + wlen],
                                    cell["y_sb"][:, 0:wlen],
                                )
                            dma_after[len(units) - 1 + y_defer] = ydma
                    out = []
                    for i, u in enumerate(units):
                        out.append(u)
                        if i in dma_after:
                            out.append(dma_after.pop(i))
                    for d in dma_after.values():  # tail stragglers
                        out.append(d)
                    return out

                n_groups = sum(
                    -(-min(S, V - v0) // 512) for v0 in range(0, V, S)
                )  # encode groups per tile
                wq = None  # decode units of the previous tile
                wq_pos = 0.0
                for t in range(nbt):
                    r0 = t * 128
                    # ---------------- encode tile t ----------------
                    ht_ps = htps.tile([D, 128], F32)
                    pending = []  # 2-group skew: slack for the evict engines
                    chunk = 0
                    gidx = 0
                    per_group = (pace * len(wq) / n_groups) if wq else 0.0
                    n_sc = -(-V // S)
                    for si, v0 in enumerate(range(0, V, S)):
                        sl = min(S, V - v0)
                        x_t = xpool.tile([128, S], BF16)
                        nc.sync.dma_start(x_t[:, 0:sl], x_d[r0 : r0 + 128, v0 : v0 + sl])
                        if t == 0:
                            # stream the encoder weights in pieces behind
                            # the x chunks they unblock (SP queue order)
                            c0 = v0 // 128
                            c1 = min(NCH, -(-(v0 + sl) // 128))
                            nc.sync.dma_start(
                                w_sb[:, c0:c1, :],
                                wenc_d[:, c0 * D : c1 * D].rearrange(
                                    "p (c d) -> p c d", d=D
                                ),
                            )
                            q0 = (si * QB) // n_sc
                            q1 = ((si + 1) * QB) // n_sc
                            if q1 > q0:
                                nc.sync.dma_start(wt_sb[:, q0:q1], wt_d[:, q0:q1])
                        for g0 in range(0, sl, 512):
                            glen = min(512, sl - g0)
                            xt_ps = xtps.tile([128, 512], BF16)
                            subs = []
                            for j, i in enumerate(range(0, glen, 128)):
                                vlen = min(128, glen - i)
                                nc.tensor.matmul(
                                    xt_ps[0:vlen, j * 128 : (j + 1) * 128],
                                    x_t[:, g0 + i : g0 + i + vlen],
                                    ident[:, 0:128],
                                    is_transpose=True,
                                )
                                subs.append((chunk, j, vlen))
                                chunk += 1
                            # evict to bf16 (values are already bf16-exact);
                            # 3:2 DVE:Pool split keeps both under PE pace
                            xt_sb = xtsb.tile([128, 512], BF16)
                            eng = nc.vector if (gidx % 5) in (0, 2, 4) else nc.gpsimd
                            nfull = sum(1 for (_, _, vl) in subs if vl == 128)
                            if nfull:
                                eng.tensor_copy(
                                    xt_sb[:, 0 : nfull * 128], xt_ps[:, 0 : nfull * 128]
                                )
                            if nfull < len(subs):
                                _, j, vl = subs[-1]
                                eng.tensor_copy(
                                    xt_sb[0:vl, j * 128 : (j + 1) * 128],
                                    xt_ps[0:vl, j * 128 : (j + 1) * 128],
                                )
                            pending.append((xt_sb, subs))
                            if len(pending) > 2:
                                _emit_accums(pending.pop(0), ht_ps)
                            # pay down the previous tile's decode units
                            if wq:
                                wq_pos += per_group
                                while wq and wq_pos >= 1.0:
                                    wq.pop(0)()
                                    wq_pos -= 1.0
                            gidx += 1
                    while pending:
                        _emit_accums(pending.pop(0), ht_ps)
                    while wq:
                        wq.pop(0)()
                    # hT = sigmoid(hT_pre + b) -> bf16, replicated to the 4
                    # PE row groups (decoder stationary)
                    ht_sb = htsb.tile([128, 128], BF16)
                    nc.scalar.activation(
                        ht_sb[0:D, :], ht_ps[:, :], SIG, bias=b_sb[:, 0:1]
                    )
                    for g in range(1, 4):
                        nc.scalar.dma_start(ht_sb[32 * g : 32 * g + D, :], ht_sb[0:D, :])
                    wq = _decode_units(t, ht_sb)
                    wq_pos = 0.0
                # drain: decode of the last tile
                while wq:
                    wq.pop(0)()

        if repeat == 1:
            _body()
        else:
            # timing aid: run the whole kernel `repeat` times on device
            # inside one NEFF (For_i back-edge ~2us per iteration)
            with tc.For_i(0, repeat, 1):
                _body()

    nc.compile()
    return nc


_NC_CACHE = None


def _get_nc():
    global _NC_CACHE
    if _NC_CACHE is None:
        _NC_CACHE = build_dae(B_CORE, V)
    return _NC_CACHE


def _prep(x, w, b):
    x_bf = np.asarray(x).astype(BF16_NP)
    w = np.ascontiguousarray(w, dtype=np.float32)
    wp = np.zeros((VPAD, D), np.float32)
    wp[:V] = w
    wenc = np.ascontiguousarray(
        wp.reshape(NCH, 128, D).transpose(1, 0, 2).reshape(128, NCH * D)
    ).astype(BF16_NP)
    wtp = np.zeros((D, 4 * QB), np.float32)
    wtp[:, :V] = w.T
    wt = np.ascontiguousarray(
        wtp.reshape(D, 4, QB).transpose(1, 0, 2).reshape(128, QB)
    ).astype(BF16_NP)
    b32 = np.ascontiguousarray(b, dtype=np.float32)
    return x_bf, wenc, wt, b32


def _in_maps(x, w, b):
    x_bf, wenc, wt, b32 = _prep(x, w, b)
    return [
        {"x": x_bf[i * B_CORE : (i + 1) * B_CORE], "wenc": wenc, "wt": wt, "b": b32}
        for i in range(N_CORES)
    ]


def kernel(x, w, b):
    assert x.shape == (B_FULL, V) and w.shape == (V, D) and b.shape == (D,)
    nc = _get_nc()
    in_maps = _in_maps(x, w, b)
    last = None
    # the first execution of a freshly compiled NEFF on this axon terminal
    # occasionally reports NRT_EXEC_UNIT_UNRECOVERABLE; a retry succeeds
    for _ in range(3):
        try:
            res = run_bass_kernel_spmd(nc, in_maps, core_ids=list(range(N_CORES)))
            break
        except Exception as e:  # noqa: BLE001
            last = e
    else:
        raise last
    y = np.concatenate([res.results[i]["y"] for i in range(N_CORES)], axis=0)
    return y.astype(np.float32)


# revision 32
# speedup vs baseline: 1.8125x; 1.8125x over previous
"""Trainium2 Bass kernel for nn_DAE_44779329028610 (embedding autoencoder).

  y = sigmoid(sigmoid(x @ w + b) @ w.T)
  x [4096, 81616] f32, w [81616, 32] f32, b [32] f32 -> y [4096, 81616] f32

Strategy: data-parallel shard of the batch dim across 8 NeuronCores
(512 rows/core); w replicated. The workload is HBM-bound, so the kernel
moves all bulk tensors in bf16 (rel-err budget is 2e-2; bf16 keeps it
~1e-3): the host pre-converts x to bf16 and pre-packs two device-friendly
w layouts (encoder layout [128, 638, 32] = w rows scattered mod 128, and
the decoder's transposed layout [128, 20480] = w.T split into 4
partition-group quarters), and y is produced in bf16 and widened on the
host. This roughly halves HBM traffic vs f32 (167+167 MB -> 84+84 MB per
core) and removes all on-device w transposition.

Per core the kernel is batch-tile pipelined: for each of the 4 tiles of
128 batch rows it encodes (stream x [128, S]-chunks; PE-transpose each
128x128 block into PSUM; evict to bf16 SBUF on DVE/Pool; accumulate
hT[32, 128] over all 638 vocab chunks with the w-chunk [128v, 32] as
stationary) and then decodes (hT sigmoid+bias on ACT, replicated to the
4 PE row groups; K=32 matmuls against the resident wT quarters at
tile_position=(32g, 0); ACT applies sigmoid PSUM->bf16 SBUF; y leaves in
[128, 4096] DMAs). Decode of tile t overlaps encode of tile t+1, so
x-read and y-write DMA streams stay concurrently busy and the ACT
sigmoid work hides under them. x reads issue on the SP DMA queue and
y writes on the ACT queue to avoid head-of-line blocking between the
two streams.
"""

import sys

if "/opt/trn_rl_repo" not in sys.path:
    sys.path.insert(0, "/opt/trn_rl_repo")

from contextlib import ExitStack

import ml_dtypes
import numpy as np

from concourse import bacc, masks, mybir, tile
from concourse.bass_utils import run_bass_kernel_spmd

# The neuronx_cc hook recompiles the NEFF from scratch in every process
# (~minutes of walrus for this kernel). Cache the compiled NEFF on disk,
# keyed by the BIR hash, so repeat runs are instant.
import hashlib
import os
import shutil

import concourse.bass2jax as _bass2jax

_NEFF_CACHE_DIR = "/tmp/bass_neff_cache"
_orig_compile_bir_kernel = _bass2jax.compile_bir_kernel


def _cached_compile_bir_kernel(bir_json, tmpdir, neff_name="file.neff"):
    os.makedirs(_NEFF_CACHE_DIR, exist_ok=True)
    key = hashlib.sha256(bir_json).hexdigest()[:32]
    cpath = os.path.join(_NEFF_CACHE_DIR, f"{key}.neff")
    out = os.path.join(tmpdir, neff_name)
    if os.path.exists(cpath):
        shutil.copyfile(cpath, out)
        return out
    out = _orig_compile_bir_kernel(bir_json, tmpdir, neff_name)
    try:
        shutil.copyfile(out, cpath)
    except OSError:
        pass
    return out


_bass2jax.compile_bir_kernel = _cached_compile_bir_kernel

F32 = mybir.dt.float32
BF16 = mybir.dt.bfloat16
F8E3 = mybir.dt.float8e3  # e3m4
BF16_NP = ml_dtypes.bfloat16
F8E3_NP = ml_dtypes.float8_e3m4
U16 = mybir.dt.uint16
SIG = mybir.ActivationFunctionType.Sigmoid
X_FP8 = True  # stream x as fp8-e3m4 (rel_l2 2.2e-3 vs 1.8e-3 for bf16)

B_FULL = 4096
V = 81616
D = 32
N_CORES = 8
B_CORE = B_FULL // N_CORES
NCH = -(-V // 128)  # 638 vocab chunks of 128
VPAD = NCH * 128  # 81664
QB = 20480  # wT quarter width (vocab cols per 32-partition group)


def build_dae(
    B_core=B_CORE,
    V=V,
    S=8192,
    YS=4096,
    x_bufs=3,
    y_bufs=4,
    y_on_sp=False,
    y_defer=0,
    pace=1.0,
    x_fp8=X_FP8,
    repeat=1,
):
    """Build + compile the per-core Bass program. S = x stream chunk,
    YS = y store chunk (one DMA each)."""
    assert B_core % 128 == 0
    nbt = B_core // 128
    assert S % 512 == 0 and YS % 1024 == 0

    nc = bacc.Bacc("TRN2", target_bir_lowering=False, debug=False)

    XDT = F8E3 if x_fp8 else BF16
    x_d = nc.dram_tensor("x", [B_core, V], XDT, kind="ExternalInput")
    wenc_d = nc.dram_tensor("wenc", [128, NCH * D], BF16, kind="ExternalInput")
    wt_d = nc.dram_tensor("wt", [128, QB], BF16, kind="ExternalInput")
    b_d = nc.dram_tensor("b", [D], F32, kind="ExternalInput")
    y_d = nc.dram_tensor("y", [B_core, V], BF16, kind="ExternalOutput")

    with tile.TileContext(nc) as tc, ExitStack() as ctx:
        const_pool = ctx.enter_context(tc.tile_pool(name="const", bufs=1))
        ident = const_pool.tile([128, 128], XDT)
        masks.make_identity(nc, ident[:])
        b_sb = const_pool.tile([D, 1], F32)
        nc.sync.dma_start(b_sb[:, 0:1], b_d[:].unsqueeze(1))
        # persistent weights: encoder layout + transposed decoder layout
        # (loaded in pieces interleaved with the first x stream -- see _body)
        w_sb = const_pool.tile([128, NCH, D], BF16)
        wt_sb = const_pool.tile([128, QB], BF16)

        def _body():
            y_eng = nc.sync if y_on_sp else nc.scalar
            with ExitStack() as es:
                xpool = es.enter_context(tc.tile_pool(name="x", bufs=x_bufs))
                xtps = es.enter_context(tc.tile_pool(name="xtps", bufs=3, space="PSUM"))
                xtsb = es.enter_context(tc.tile_pool(name="xtsb", bufs=4))
                htps = es.enter_context(tc.tile_pool(name="htps", bufs=1, space="PSUM"))
                htsb = es.enter_context(tc.tile_pool(name="htsb", bufs=2))
                yps = es.enter_context(tc.tile_pool(name="yps", bufs=2, space="PSUM"))
                ysb = es.enter_context(tc.tile_pool(name="ysb", bufs=y_bufs))

                def _emit_accums(pending, ht_ps):
                    xt_sb, subs = pending
                    for ci, j, vlen in subs:
                        if x_fp8:
                            mov = xt_sb[0:vlen, j, :, 0]
                        else:
                            mov = xt_sb[0:vlen, j * 128 : (j + 1) * 128]
                        nc.tensor.matmul(
                            ht_ps[:, :],
                            w_sb[0:vlen, ci, :],
                            mov,
                            start=(ci == 0),
                            stop=(ci == NCH - 1),
                        )

                def _decode_units(t, ht_sb):
                    """Decode tile t as a list of closures, each: 2 matmuls
                    into a fresh y_ps + 1 ACT sigmoid (+ y DMA on the last
                    piece of a y_sb). Interleaved into the NEXT tile's encode
                    emission so the in-order PE queue never couples encode
                    progress to the ACT-paced PSUM drain."""
                    r0 = t * 128
                    units = []
                    dma_after = {}  # unit index -> y DMA closure (deferred)
                    for g in range(4):
                        q0 = g * QB
                        qvalid = min(QB, V - q0)
                        for yb0 in range(0, qvalid, YS):
                            wlen = min(YS, qvalid - yb0)
                            ns0 = -(-wlen // 1024) * 1024
                            cell = {}
                            for s0 in range(0, ns0, 1024):
                                def u(g=g, yb0=yb0, s0=s0, cell=cell, ht_sb=ht_sb):
                                    if s0 == 0:
                                        cell["y_sb"] = ysb.tile(
                                            [128, YS], BF16, name="y_sb"
                                        )
                                    y_sb = cell["y_sb"]
                                    y_ps = yps.tile([128, 1024], F32)
                                    for m0 in range(0, 1024, 512):
                                        nc.tensor.matmul(
                                            y_ps[:, m0 : m0 + 512],
                                            ht_sb[32 * g : 32 * g + D, :],
                                            wt_sb[
                                                32 * g : 32 * g + D,
                                                yb0 + s0 + m0 : yb0 + s0 + m0 + 512,
                                            ],
                                            tile_position=(32 * g, 0),
                                        )
                                    nc.scalar.activation(
                                        y_sb[:, s0 : s0 + 1024], y_ps[:, :], SIG
                                    )
                                units.append(u)
                            # y write issued a few units after its last
                            # sigmoid so its semaphore wait is already
                            # satisfied when the sequencer reaches it
                            def ydma(q0=q0, yb0=yb0, wlen=wlen, cell=cell, r0=r0):
                                y_eng.dma_start(
                                    y_d[r0 : r0 + 128, q0 + yb0 : q0 + yb0 + wlen],
                                    cell["y_sb"][:, 0:wlen],
                                )
                            dma_after[len(units) - 1 + y_defer] = ydma
                    out = []
                    for i, u in enumerate(units):
                        out.append(u)
                        if i in dma_after:
                            out.append(dma_after.pop(i))
                    for d in dma_after.values():  # tail stragglers
                        out.append(d)
                    return out

                n_groups = sum(
                    -(-min(S, V - v0) // 512) for v0 in range(0, V, S)
                )  # encode groups per tile
                wq = None  # decode units of the previous tile
                wq_pos = 0.0
                for t in range(nbt):
                    r0 = t * 128
                    # ---------------- encode tile t ----------------
                    ht_ps = htps.tile([D, 128], F32)
                    pending = []  # 2-group skew: slack for the evict engines
                    chunk = 0
                    gidx = 0
                    per_group = (pace * len(wq) / n_groups) if wq else 0.0
                    n_sc = -(-V // S)
                    for si, v0 in enumerate(range(0, V, S)):
                        sl = min(S, V - v0)
                        x_t = xpool.tile([128, S], XDT)
                        nc.sync.dma_start(x_t[:, 0:sl], x_d[r0 : r0 + 128, v0 : v0 + sl])
                        if t == 0:
                            # stream the encoder weights in pieces behind
                            # the x chunks they unblock (SP queue order)
                            c0 = v0 // 128
                            c1 = min(NCH, -(-(v0 + sl) // 128))
                            nc.sync.dma_start(
                                w_sb[:, c0:c1, :],
                                wenc_d[:, c0 * D : c1 * D].rearrange(
                                    "p (c d) -> p c d", d=D
                                ),
                            )
                            q0 = (si * QB) // n_sc
                            q1 = ((si + 1) * QB) // n_sc
                            if q1 > q0:
                                nc.sync.dma_start(wt_sb[:, q0:q1], wt_d[:, q0:q1])
                        for g0 in range(0, sl, 512):
                            glen = min(512, sl - g0)
                            # fp8 transpose mode writes its output with an
                            # element step of 2 bytes (HW pads fp8 to 16 bit),
                            # so the fp8 PSUM/SBUF tiles carry an explicit
                            # trailing stride-2 dim; evictions copy the raw
                            # 2-byte cells (uint16 view -> DVE fast path) and
                            # the accumulate reads the strided fp8 view.
                            if x_fp8:
                                xt_ps = xtps.tile([128, 4, 128, 2], XDT)
                                xt_sb = xtsb.tile([128, 4, 128, 2], XDT)
                            else:
                                xt_ps = xtps.tile([128, 512], XDT)
                                xt_sb = xtsb.tile([128, 512], XDT)
                            subs = []
                            for j, i in enumerate(range(0, glen, 128)):
                                vlen = min(128, glen - i)
                                out_ap = (
                                    xt_ps[0:vlen, j, :, 0]
                                    if x_fp8
                                    else xt_ps[0:vlen, j * 128 : (j + 1) * 128]
                                )
                                nc.tensor.matmul(
                                    out_ap,
                                    x_t[:, g0 + i : g0 + i + vlen],
                                    ident[:, 0:128],
                                    is_transpose=True,
                                )
                                subs.append((chunk, j, vlen))
                                chunk += 1
                            # evict on DVE (Pool/GPSIMD cannot access PSUM)
                            eng = nc.vector

                            def _evict(p0, p1, j0, j1):
                                if x_fp8:
                                    eng.tensor_copy(
                                        xt_sb[p0:p1, j0:j1, :, :].bitcast(U16),
                                        xt_ps[p0:p1, j0:j1, :, :].bitcast(U16),
                                    )
                                else:
                                    eng.tensor_copy(
                                        xt_sb[p0:p1, j0 * 128 : j1 * 128],
                                        xt_ps[p0:p1, j0 * 128 : j1 * 128],
                                    )

                            nfull = sum(1 for (_, _, vl) in subs if vl == 128)
                            if nfull:
                                _evict(0, 128, 0, nfull)
                            if nfull < len(subs):
                                _, j, vl = subs[-1]
                                _evict(0, vl, j, j + 1)
                            pending.append((xt_sb, subs))
                            if len(pending) > 2:
                                _emit_accums(pending.pop(0), ht_ps)
                            # pay down the previous tile's decode units
                            if wq:
                                wq_pos += per_group
                                while wq and wq_pos >= 1.0:
                                    wq.pop(0)()
                                    wq_pos -= 1.0
                            gidx += 1
                    while pending:
                        _emit_accums(pending.pop(0), ht_ps)
                    while wq:
                        wq.pop(0)()
                    # hT = sigmoid(hT_pre + b) -> bf16, replicated to the 4
                    # PE row groups (decoder stationary)
                    ht_sb = htsb.tile([128, 128], BF16)
                    nc.scalar.activation(
                        ht_sb[0:D, :], ht_ps[:, :], SIG, bias=b_sb[:, 0:1]
                    )
                    for g in range(1, 4):
                        nc.scalar.dma_start(ht_sb[32 * g : 32 * g + D, :], ht_sb[0:D, :])
                    wq = _decode_units(t, ht_sb)
                    wq_pos = 0.0
                # drain: decode of the last tile
                while wq:
                    wq.pop(0)()

        if repeat == 1:
            _body()
        else:
            # timing aid: run the whole kernel `repeat` times on device
            # inside one NEFF (For_i back-edge ~2us per iteration)
            with tc.For_i(0, repeat, 1):
                _body()

    nc.compile()
    return nc


_NC_CACHE = None


def _get_nc():
    global _NC_CACHE
    if _NC_CACHE is None:
        _NC_CACHE = build_dae(B_CORE, V)
    return _NC_CACHE


def _prep(x, w, b):
    x_bf = np.asarray(x).astype(F8E3_NP if X_FP8 else BF16_NP)
    w = np.ascontiguousarray(w, dtype=np.float32)
    wp = np.zeros((VPAD, D), np.float32)
    wp[:V] = w
    wenc = np.ascontiguousarray(
        wp.reshape(NCH, 128, D).transpose(1, 0, 2).reshape(128, NCH * D)
    ).astype(BF16_NP)
    wtp = np.zeros((D, 4 * QB), np.float32)
    wtp[:, :V] = w.T
    wt = np.ascontiguousarray(
        wtp.reshape(D, 4, QB).transpose(1, 0, 2).reshape(128, QB)
    ).astype(BF16_NP)
    b32 = np.ascontiguousarray(b, dtype=np.float32)
    return x_bf, wenc, wt, b32


def _in_maps(x, w, b):
    x_bf, wenc, wt, b32 = _prep(x, w, b)
    return [
        {"x": x_bf[i * B_CORE : (i + 1) * B_CORE], "wenc": wenc, "wt": wt, "b": b32}
        for i in range(N_CORES)
    ]


def kernel(x, w, b):
    assert x.shape == (B_FULL, V) and w.shape == (V, D) and b.shape == (D,)
    nc = _get_nc()
    in_maps = _in_maps(x, w, b)
    last = None
    # the first execution of a freshly compiled NEFF on this axon terminal
    # occasionally reports NRT_EXEC_UNIT_UNRECOVERABLE; a retry succeeds
    for _ in range(3):
        try:
            res = run_bass_kernel_spmd(nc, in_maps, core_ids=list(range(N_CORES)))
            break
        except Exception as e:  # noqa: BLE001
            last = e
    else:
        raise last
    y = np.concatenate([res.results[i]["y"] for i in range(N_CORES)], axis=0)
    return y.astype(np.float32)
